# revision 12
# baseline (speedup 1.0000x reference)
"""DAWNLayer Trainium2 kernel (8-core SPMD).

Sharding: expert-parallel over N (fe_w2 columns) for the big feature matmul
with an all-reduce on the partial intermediate; expert-parallel over P (wt)
for the per-neuron transform with an all-reduce on the partial combined.
All token-wise small work (input acts, proc acts, layernorms) is replicated.
The program is identical on all 8 cores; shard selection happens via
per-core input data (weight shards + one-hot selector matrices).
"""

import sys

sys.path.insert(0, "/opt/trn_rl_repo")

import numpy as np
from contextlib import ExitStack

import concourse.bass as bass
import concourse.bacc as bacc
import concourse.tile as tile
from concourse import mybir
from concourse.bass_utils import run_bass_kernel_spmd
from concourse.masks import make_identity

F32 = mybir.dt.float32
F16 = mybir.dt.float16
AF = mybir.ActivationFunctionType
ALU = mybir.AluOpType

B, S, H, N, P = 4, 512, 512, 64, 128
T = B * S          # 2048 tokens
FH = 4 * H         # 2048
NC = 8             # cores
E = N // NC        # 8 experts (n) per core
Q = P // NC        # 16 process-neurons (p) per core
TT = T // 128      # 16 token tiles
KT = FH // 128     # 16 k tiles for the big matmul
HT = H // 128      # 4 h tiles
LN_EPS = 1e-5

# token groups for the big matmul / all-reduce chunking
NG = 4             # groups
GTT = TT // NG     # 4 token tiles per group


def _emit(nc, tc, io):
    ctx = ExitStack()
    with ctx:
        _emit_inner(nc, tc, ctx, io)


def _emit_inner(nc, tc, ctx, io):
    # ---------- long-lived pools ----------
    res = ctx.enter_context(tc.tile_pool(name="res", bufs=1))
    ps_small = ctx.enter_context(tc.tile_pool(name="ps_small", bufs=2, space="PSUM"))
    dram = ctx.enter_context(tc.tile_pool(name="dram", bufs=1, space="DRAM"))

    ident = res.tile([128, 128], F32, tag="ident")
    make_identity(nc, ident)

    # long-lived intermediates
    hid_es = ExitStack()
    hid_pool = hid_es.enter_context(tc.tile_pool(name="hid", bufs=1))
    hidT = hid_pool.tile([128, KT, T], F16, tag="hidT")      # 64KB/part
    acts_sh = res.tile([128, TT, E], F32, tag="acts_sh")     # raw acts, my experts
    pa = res.tile([128, TT, P], F32, tag="pa")               # proc_acts token-major
    paT_sh = res.tile([Q, T], F32, tag="paT_sh")             # my p rows, token free
    actsT_sh = res.tile([E, T], F32, tag="actsT_sh")         # my n rows (raw acts)
    selp_sb = res.tile([128, Q], F32, tag="selp")
    b2_sb = res.tile([E, H], F32, tag="b2")
    nc.sync.dma_start(out=selp_sb, in_=io["selp"][:, :])
    nc.sync.dma_start(out=b2_sb, in_=io["b2s"][:, :])

    # collective bounce buffers (DRAM)
    cc1_in = [dram.tile([GTT * 128, H], F32, tag=f"cc1i{g}", name=f"cc1i{g}") for g in range(NG)]
    cc1_out = [dram.tile([GTT * 128, H], F32, tag=f"cc1o{g}", name=f"cc1o{g}", addr_space="Shared") for g in range(NG)]
    cc2_in = [dram.tile([GTT * 128, H], F32, tag=f"cc2i{g}", name=f"cc2i{g}") for g in range(NG)]
    cc2_out = [dram.tile([GTT * 128, H], F32, tag=f"cc2o{g}", name=f"cc2o{g}", addr_space="Shared") for g in range(NG)]
    groups = [list(range(NC))]

    # ================= phase 1: acts, hidT, proc_acts, shard selects ==========
    ph1 = ExitStack()
    p1 = ph1.enter_context(tc.tile_pool(name="p1", bufs=1))
    p1s = ph1.enter_context(tc.tile_pool(name="p1s", bufs=3))
    ps1 = ph1.enter_context(tc.tile_pool(name="ps1", bufs=1, space="PSUM"))

    xTh = p1.tile([128, HT, T], F16, tag="xTh")
    w1_sb = p1.tile([128, HT, FH], F16, tag="w1")
    b1_sb = p1.tile([128, KT], F32, tag="b1")
    nc.sync.dma_start(out=xTh, in_=io["xTh"].rearrange("(a p) t -> p a t", p=128))
    nc.sync.dma_start(out=w1_sb, in_=io["w1h"].rearrange("(a p) c -> p a c", p=128))
    b1d = io["b1"]
    nc.sync.dma_start(
        out=b1_sb,
        in_=bass.AP(tensor=b1d.tensor, offset=b1d.offset, ap=[[1, 128], [128, KT]]),
    )

    # 1/||x|| per token
    inv_xn = p1.tile([128, TT], F32, tag="inv_xn")
    for tt in range(TT):
        xt = p1s.tile([128, H], F32, tag="xt")
        nc.sync.dma_start(out=xt, in_=io["x"][tt * 128:(tt + 1) * 128, :])
        sq = p1s.tile([128, H], F32, tag="sq")
        ssq = p1s.tile([128, 1], F32, tag="ssq")
        nc.scalar.activation(sq, xt, AF.Square, accum_out=ssq)
        nrm = p1s.tile([128, 1], F32, tag="nrm")
        nc.scalar.activation(nrm, ssq, AF.Sqrt)
        nc.vector.tensor_scalar_max(nrm, nrm, 1e-12)
        nc.vector.reciprocal(inv_xn[:, tt:tt + 1], nrm)

    # normalized patterns, transposed -> p_nT (HT tiles of (128,N)) fp16
    pat = p1.tile([N, H], F32, tag="pat")
    nc.sync.dma_start(out=pat, in_=io["pat"][:, :])
    psq = p1.tile([N, H], F32, tag="psq")
    pss = p1.tile([N, 1], F32, tag="pss")
    nc.scalar.activation(psq, pat, AF.Square, accum_out=pss)
    nc.scalar.activation(pss, pss, AF.Sqrt)
    nc.vector.tensor_scalar_max(pss, pss, 1e-12)
    pinv = p1.tile([N, 1], F32, tag="pinv")
    nc.vector.reciprocal(pinv, pss)
    p_n = p1.tile([N, H], F32, tag="p_n")
    nc.vector.tensor_scalar_mul(p_n, pat, pinv)
    p_nT = p1.tile([128, HT, N], F16, tag="p_nT")
    for hi in range(HT):
        tp = ps1.tile([128, N], F32, tag="ps_sm", bufs=3)
        nc.tensor.transpose(tp, p_n[:, hi * 128:(hi + 1) * 128], ident[:N, :N])
        nc.scalar.activation(p_nT[:, hi, :], tp, AF.Copy)

    # input acts (token-major) + transpose to actsT; also a_n and a_nT
    actsT = p1.tile([N, T], F32, tag="actsT")
    a_nT = p1.tile([N, T], F32, tag="a_nT")
    for tt in range(TT):
        ps_l = ps1.tile([128, N], F32, tag="ps_sm", bufs=3)
        for hi in range(HT):
            nc.tensor.matmul(
                ps_l,
                lhsT=xTh[:, hi, tt * 128:(tt + 1) * 128],
                rhs=p_nT[:, hi, :],
                start=(hi == 0),
                stop=(hi == HT - 1),
            )
        at = p1s.tile([128, N], F32, tag="at")
        nc.scalar.activation(at, ps_l, AF.Sigmoid, scale=inv_xn[:, tt:tt + 1])
        nc.sync.dma_start(out=io["input_acts"][tt * 128:(tt + 1) * 128, :], in_=at)
        # transpose raw acts
        tpa = ps1.tile([N, 128], F32, tag="ps_sm", bufs=3)
        nc.tensor.transpose(tpa, at, ident)
        nc.scalar.activation(actsT[:, tt * 128:(tt + 1) * 128], tpa, AF.Copy)
        # a_n = acts / ||acts||_2 (over N, free dim), then transpose
        asq = p1s.tile([128, N], F32, tag="asq")
        ass = p1s.tile([128, 1], F32, tag="ass")
        nc.scalar.activation(asq, at, AF.Square, accum_out=ass)
        nc.scalar.activation(ass, ass, AF.Sqrt)
        nc.vector.tensor_scalar_max(ass, ass, 1e-12)
        ainv = p1s.tile([128, 1], F32, tag="ainv")
        nc.vector.reciprocal(ainv, ass)
        an_t = p1s.tile([128, N], F32, tag="an_t")
        nc.vector.tensor_scalar_mul(an_t, at, ainv)
        tpn = ps1.tile([N, 128], F32, tag="ps_sm", bufs=3)
        nc.tensor.transpose(tpn, an_t, ident)
        nc.scalar.activation(a_nT[:, tt * 128:(tt + 1) * 128], tpn, AF.Copy)

    # shard selects from actsT: acts_sh (token-major cols) and actsT_sh (rows)
    seln_sb = p1.tile([N, E], F32, tag="seln")
    nc.sync.dma_start(out=seln_sb, in_=io["seln"][:, :])
    for tt in range(TT):
        pssel = ps1.tile([128, E], F32, tag="ps_sm", bufs=3)
        nc.tensor.matmul(
            pssel, lhsT=actsT[:, tt * 128:(tt + 1) * 128], rhs=seln_sb,
            start=True, stop=True,
        )
        nc.scalar.activation(acts_sh[:, tt, :], pssel, AF.Copy)
    for c in range(4):
        psr = ps1.tile([E, 512], F32, tag="ps_sm", bufs=3)
        nc.tensor.matmul(
            psr, lhsT=seln_sb, rhs=actsT[:, c * 512:(c + 1) * 512],
            start=True, stop=True,
        )
        nc.scalar.activation(actsT_sh[:, c * 512:(c + 1) * 512], psr, AF.Copy)

    # normalized templates -> t_nT (N x P) f32
    tmpl = p1.tile([P, N], F32, tag="tmpl")
    nc.sync.dma_start(out=tmpl, in_=io["tmpl"][:, :])
    tsq = p1.tile([P, N], F32, tag="tsq")
    tss = p1.tile([P, 1], F32, tag="tss")
    nc.scalar.activation(tsq, tmpl, AF.Square, accum_out=tss)
    nc.scalar.activation(tss, tss, AF.Sqrt)
    nc.vector.tensor_scalar_max(tss, tss, 1e-12)
    tinv = p1.tile([P, 1], F32, tag="tinv")
    nc.vector.reciprocal(tinv, tss)
    t_n = p1.tile([P, N], F32, tag="t_n")
    nc.vector.tensor_scalar_mul(t_n, tmpl, tinv)
    t_nT = p1.tile([N, P], F32, tag="t_nT")
    tpt = ps1.tile([N, P], F32, tag="ps_sm", bufs=3)
    nc.tensor.transpose(tpt, t_n, ident)
    nc.scalar.activation(t_nT, tpt, AF.Copy)

    # proc_acts: sigmoid(a_n @ t_n.T), token-major; plus transpose + p-shard rows
    paT = p1.tile([128, T], F32, tag="paT")
    for tt in range(TT):
        ps_p = ps1.tile([128, P], F32, tag="ps_sm", bufs=3)
        nc.tensor.matmul(
            ps_p, lhsT=a_nT[:, tt * 128:(tt + 1) * 128], rhs=t_nT,
            start=True, stop=True,
        )
        nc.scalar.activation(pa[:, tt, :], ps_p, AF.Sigmoid)
        nc.sync.dma_start(out=io["proc_acts"][tt * 128:(tt + 1) * 128, :], in_=pa[:, tt, :])
        tpq = ps1.tile([128, 128], F32, tag="ps_sm", bufs=3)
        nc.tensor.transpose(tpq, pa[:, tt, :], ident)
        nc.scalar.activation(paT[:, tt * 128:(tt + 1) * 128], tpq, AF.Copy)
    for c in range(4):
        psq2 = ps1.tile([Q, 512], F32, tag="ps_sm", bufs=3)
        nc.tensor.matmul(
            psq2, lhsT=selp_sb, rhs=paT[:, c * 512:(c + 1) * 512],
            start=True, stop=True,
        )
        nc.scalar.activation(paT_sh[:, c * 512:(c + 1) * 512], psq2, AF.Copy)

    # hidT = gelu(w1.T @ x.T + b1) in fp16, (FH x T) as [128, KT, T]
    for ci in range(KT):
        for tj in range(T // 512):
            ps_h = ps1.tile([128, 512], F32, tag="ps_h", bufs=3)
            for hi in range(HT):
                nc.tensor.matmul(
                    ps_h,
                    lhsT=w1_sb[:, hi, ci * 128:(ci + 1) * 128],
                    rhs=xTh[:, hi, tj * 512:(tj + 1) * 512],
                    start=(hi == 0),
                    stop=(hi == HT - 1),
                )
            nc.scalar.activation(
                hidT[:, ci, tj * 512:(tj + 1) * 512], ps_h, AF.Gelu,
                bias=b1_sb[:, ci:ci + 1],
            )

    ph1.close()

    # ================= phase 2: big matmul + weighted reduce + AR1 ===========
    ph2 = ExitStack()
    accp = ph2.enter_context(tc.tile_pool(name="accp", bufs=1))
    w2p = ph2.enter_context(tc.tile_pool(name="w2p", bufs=3))
    featp = ph2.enter_context(tc.tile_pool(name="featp", bufs=6, space="PSUM"))

    acc = [accp.tile([128, H], F32, tag=f"acc{t}", name=f"acc{t}") for t in range(TT)]
    # init acc with the b2 term: acts_shard @ b2_shard
    for tt in range(TT):
        psb = ps_small.tile([128, H], F32, tag="psb")
        nc.tensor.matmul(
            psb, lhsT=actsT_sh[:, tt * 128:(tt + 1) * 128], rhs=b2_sb,
            start=True, stop=True,
        )
        nc.scalar.activation(acc[tt], psb, AF.Copy)

    w2d = io["w2h"]
    for g in range(NG):
        for n in range(E):
            # one 2MB DMA: (FH, 512) slice of my w2 shard, k-tiled on partitions
            w2t = w2p.tile([128, KT, 512], F16, tag="w2t")
            src = bass.AP(
                tensor=w2d.tensor,
                offset=w2d.offset + n * 512,
                ap=[[E * H, 128], [128 * E * H, KT], [1, 512]],
            )
            nc.sync.dma_start(out=w2t, in_=src)
            fps = []
            for tt in range(GTT):
                t_abs = g * GTT + tt
                fp = featp.tile([128, 512], F32, tag="fp")
                fps.append(fp)
                for ki in range(KT):
                    nc.tensor.matmul(
                        fp,
                        lhsT=hidT[:, ki, t_abs * 128:(t_abs + 1) * 128],
                        rhs=w2t[:, ki, :],
                        start=(ki == 0),
                        stop=(ki == KT - 1),
                    )
            for tt in range(GTT):
                t_abs = g * GTT + tt
                nc.vector.scalar_tensor_tensor(
                    out=acc[t_abs],
                    in0=fps[tt],
                    scalar=acts_sh[:, t_abs, n:n + 1],
                    in1=acc[t_abs],
                    op0=ALU.mult,
                    op1=ALU.add,
                )
        # group done -> stage to DRAM and all-reduce
        for tt in range(GTT):
            t_abs = g * GTT + tt
            nc.sync.dma_start(out=cc1_in[g][tt * 128:(tt + 1) * 128, :], in_=acc[t_abs])
        nc.gpsimd.collective_compute(
            "AllReduce", ALU.add, replica_groups=groups,
            ins=[cc1_in[g][:, :]], outs=[cc1_out[g][:, :]],
        )

    ph2.close()
    hid_es.close()

    # ================= phase 3/4: x1, process neurons, combined, output ======
    ph4 = ExitStack()
    p4 = ph4.enter_context(tc.tile_pool(name="p4", bufs=1))
    p4s = ph4.enter_context(tc.tile_pool(name="p4s", bufs=4))
    ps4 = ph4.enter_context(tc.tile_pool(name="ps4", bufs=1, space="PSUM"))
    wtp = ph4.enter_context(tc.tile_pool(name="wtp", bufs=3))

    # broadcast LN params to all partitions
    lnp = {}
    for nm in ("g1", "bb1", "g2", "bb2"):
        t = p4.tile([128, H], F32, tag=nm)
        d = io[nm]
        nc.sync.dma_start(
            out=t, in_=bass.AP(tensor=d.tensor, offset=d.offset, ap=[[0, 128], [1, H]])
        )
        lnp[nm] = t

    def layer_norm(dst, src, gt, bt):
        st = p4s.tile([128, nc.vector.BN_STATS_DIM], F32, tag="st")
        mv = p4s.tile([128, nc.vector.BN_AGGR_DIM], F32, tag="mv")
        nc.vector.bn_stats(out=st, in_=src)
        nc.vector.bn_aggr(out=mv, in_=st)
        sd = p4s.tile([128, 1], F32, tag="sd")
        nc.vector.tensor_scalar_add(sd, mv[:, 1:2], LN_EPS)
        nc.scalar.activation(sd, sd, AF.Sqrt)
        rin = p4s.tile([128, 1], F32, tag="rin")
        nc.vector.reciprocal(rin, sd)
        nc.vector.tensor_scalar(
            out=dst, in0=src, scalar1=mv[:, 0:1], scalar2=rin,
            op0=ALU.subtract, op1=ALU.mult,
        )
        nc.vector.tensor_mul(dst, dst, gt)
        nc.vector.tensor_add(dst, dst, bt)

    # x1 = LN(x + intermediate)
    x1 = p4.tile([128, TT, H], F32, tag="x1")
    for g in range(NG):
        for tt in range(GTT):
            t_abs = g * GTT + tt
            it = p4s.tile([128, H], F32, tag="it")
            nc.sync.dma_start(out=it, in_=cc1_out[g][tt * 128:(tt + 1) * 128, :])
            xt2 = p4s.tile([128, H], F32, tag="xt2")
            nc.sync.dma_start(out=xt2, in_=io["x"][t_abs * 128:(t_abs + 1) * 128, :])
            pre = p4s.tile([128, H], F32, tag="pre")
            nc.vector.tensor_add(pre, xt2, it)
            layer_norm(x1[:, t_abs, :], pre, lnp["g1"], lnp["bb1"])

    # contexts (all p, per batch) + norms; then select my Q rows
    ones = p4.tile([128, 1], F32, tag="ones")
    nc.vector.memset(ones, 1.0)
    ctxTq = p4.tile([128, HT, B, Q], F32, tag="ctxTq")  # h-part, (b, my p) free
    for b in range(B):
        ps_c = ps4.tile([P, H], F32, tag="ps_big", bufs=3)
        ps_n = ps4.tile([P, 1], F32, tag="ps_sm4", bufs=3)
        for st in range(4):
            t_abs = b * 4 + st
            nc.tensor.matmul(
                ps_c, lhsT=pa[:, t_abs, :], rhs=x1[:, t_abs, :],
                start=(st == 0), stop=(st == 3),
            )
        for st in range(4):
            nc.tensor.matmul(
                ps_n, lhsT=pa[:, b * 4 + st, :], rhs=ones,
                start=(st == 0), stop=(st == 3),
            )
        ctx_f = p4s.tile([P, H], F32, tag="ctx_f")
        nc.scalar.activation(ctx_f, ps_c, AF.Copy)
        nrm_f = p4s.tile([P, 1], F32, tag="nrm_f")
        nc.vector.tensor_scalar_add(nrm_f, ps_n, 1e-8)
        # select my Q rows of ctx and norm
        ps_cs = ps4.tile([Q, H], F32, tag="ps_sm4", bufs=3)
        nc.tensor.matmul(ps_cs, lhsT=selp_sb, rhs=ctx_f, start=True, stop=True)
        ps_ns = ps4.tile([Q, 1], F32, tag="ps_sm4", bufs=3)
        nc.tensor.matmul(ps_ns, lhsT=selp_sb, rhs=nrm_f, start=True, stop=True)
        rn = p4s.tile([Q, 1], F32, tag="rn")
        nc.vector.reciprocal(rn, ps_ns)
        ctx_s = p4s.tile([Q, H], F32, tag="ctx_s")
        nc.vector.tensor_scalar_mul(ctx_s, ps_cs, rn)
        # transpose to (h, q) blocks
        for hi in range(HT):
            tpc = ps4.tile([128, Q], F32, tag="ps_sm4", bufs=3)
            nc.tensor.transpose(tpc, ctx_s[:, hi * 128:(hi + 1) * 128], ident[:Q, :Q])
            nc.scalar.activation(ctxTq[:, hi, b, :], tpc, AF.Copy)

    # per-neuron transform: out[b,k] = sum_h ctx[b,q,h] * wtT[q,h,k]
    trans = p4.tile([Q, B, H], F32, tag="trans")  # q-part, (b, k) free
    wtd = io["wtT"]
    for q in range(Q):
        wtt = wtp.tile([128, HT, H], F32, tag="wtt")
        src = bass.AP(
            tensor=wtd.tensor,
            offset=wtd.offset + q * H * H,
            ap=[[H, 128], [128 * H, HT], [1, H]],
        )
        nc.sync.dma_start(out=wtt, in_=src)
        ps_t = ps4.tile([B, H], F32, tag="ps_sm4", bufs=3)
        for hi in range(HT):
            nc.tensor.matmul(
                ps_t,
                lhsT=ctxTq[:, hi, :, q],
                rhs=wtt[:, hi, :],
                start=(hi == 0),
                stop=(hi == HT - 1),
            )
        t4 = p4s.tile([B, H], F32, tag="t4")
        nc.scalar.activation(t4, ps_t, AF.Copy)
        # partition->free scatter: DMA the (B,H) block into one partition row
        nc.sync.dma_start(out=trans[q:q + 1, :, :], in_=t4)

    # combined partial = paT_sh.T @ trans  (token-major), then AR2 + final LN
    for g in range(NG):
        for tt in range(GTT):
            t_abs = g * GTT + tt
            b = t_abs // 4
            ps_m = ps4.tile([128, H], F32, tag="ps_big", bufs=3)
            nc.tensor.matmul(
                ps_m,
                lhsT=paT_sh[:, t_abs * 128:(t_abs + 1) * 128],
                rhs=trans[:, b, :],
                start=True, stop=True,
            )
            cmb = p4s.tile([128, H], F32, tag="cmb")
            nc.scalar.activation(cmb, ps_m, AF.Copy)
            nc.sync.dma_start(out=cc2_in[g][tt * 128:(tt + 1) * 128, :], in_=cmb)
        nc.gpsimd.collective_compute(
            "AllReduce", ALU.add, replica_groups=groups,
            ins=[cc2_in[g][:, :]], outs=[cc2_out[g][:, :]],
        )

    for g in range(NG):
        for tt in range(GTT):
            t_abs = g * GTT + tt
            cf = p4s.tile([128, H], F32, tag="cf")
            nc.sync.dma_start(out=cf, in_=cc2_out[g][tt * 128:(tt + 1) * 128, :])
            pre2 = p4s.tile([128, H], F32, tag="pre2")
            nc.vector.tensor_add(pre2, x1[:, t_abs, :], cf)
            ot = p4s.tile([128, H], F32, tag="ot")
            layer_norm(ot, pre2, lnp["g2"], lnp["bb2"])
            nc.sync.dma_start(out=io["out"][t_abs * 128:(t_abs + 1) * 128, :], in_=ot)

    ph4.close()


def _build_program():
    nc = bacc.Bacc("TRN2", target_bir_lowering=False, debug=False, num_devices=NC)
    io = {}

    def din(name, shape, dt):
        io[name] = nc.dram_tensor(name, list(shape), dt, kind="ExternalInput").ap()

    def dout(name, shape, dt):
        io[name] = nc.dram_tensor(name, list(shape), dt, kind="ExternalOutput").ap()

    din("x", (T, H), F32)
    din("xTh", (H, T), F16)
    din("w1h", (H, FH), F16)
    din("b1", (FH,), F32)
    din("w2h", (FH, E * H), F16)
    din("b2s", (E, H), F32)
    din("pat", (N, H), F32)
    din("tmpl", (P, N), F32)
    din("wtT", (Q, H, H), F32)
    din("seln", (N, E), F32)
    din("selp", (P, Q), F32)
    for nm in ("g1", "bb1", "g2", "bb2"):
        din(nm, (H,), F32)
    dout("out", (T, H), F32)
    dout("input_acts", (T, N), F32)
    dout("proc_acts", (T, P), F32)

    with tile.TileContext(nc) as tc:
        _emit(nc, tc, io)
    nc.compile()
    return nc


def kernel(x, patterns, fe_w1, fe_b1, fe_w2, fe_b2, templates, wt,
           ln1_g, ln1_b, ln2_g, ln2_b):
    in_maps = _prep_in_maps(x, patterns, fe_w1, fe_b1, fe_w2, fe_b2, templates,
                            wt, ln1_g, ln1_b, ln2_g, ln2_b)
    nc = _build_program()
    res = run_bass_kernel_spmd(nc, in_maps, core_ids=list(range(NC)))
    r0 = res.results[0]
    return (
        r0["out"].reshape(B, S, H).astype(np.float32),
        r0["input_acts"].reshape(B, S, N).astype(np.float32),
        r0["proc_acts"].reshape(B, S, P).astype(np.float32),
    )


def _prep_in_maps(x, patterns, fe_w1, fe_b1, fe_w2, fe_b2, templates, wt,
                  ln1_g, ln1_b, ln2_g, ln2_b):
    x = np.asarray(x, np.float32)
    x2 = np.ascontiguousarray(x.reshape(T, H))
    xT16 = np.ascontiguousarray(x2.T).astype(np.float16)
    w1h = np.asarray(fe_w1, np.float32).astype(np.float16)
    w2 = np.asarray(fe_w2, np.float32)
    b2 = np.asarray(fe_b2, np.float32).reshape(N, H)
    wtf = np.asarray(wt, np.float32)

    common = dict(
        x=x2, xTh=xT16, w1h=np.ascontiguousarray(w1h),
        b1=np.asarray(fe_b1, np.float32),
        pat=np.asarray(patterns, np.float32),
        tmpl=np.asarray(templates, np.float32),
        g1=np.asarray(ln1_g, np.float32), bb1=np.asarray(ln1_b, np.float32),
        g2=np.asarray(ln2_g, np.float32), bb2=np.asarray(ln2_b, np.float32),
    )
    in_maps = []
    for c in range(NC):
        n0, p0 = c * E, c * Q
        seln = np.zeros((N, E), np.float32)
        seln[n0 + np.arange(E), np.arange(E)] = 1.0
        selp = np.zeros((P, Q), np.float32)
        selp[p0 + np.arange(Q), np.arange(Q)] = 1.0
        m = dict(common)
        m["w2h"] = np.ascontiguousarray(
            w2.reshape(FH, N, H)[:, n0:n0 + E, :].reshape(FH, E * H)
        ).astype(np.float16)
        m["b2s"] = np.ascontiguousarray(b2[n0:n0 + E])
        m["wtT"] = np.ascontiguousarray(wtf[p0:p0 + Q].transpose(0, 2, 1))
        m["seln"] = seln
        m["selp"] = selp
        in_maps.append(m)
    return in_maps


def make_in_maps(inputs):
    """Build the per-core in_maps from full reference inputs (for profiling)."""
    import inspect
    sig = inspect.signature(kernel)
    return _prep_in_maps(**{k: inputs[k] for k in sig.parameters})


def traced_run(inputs, trace_cores=None):
    in_maps = make_in_maps(inputs)
    nc = _build_program()
    res = run_bass_kernel_spmd(
        nc, in_maps, core_ids=list(range(NC)), trace=True,
        trace_cores=trace_cores,
    )
    return res
